# revision 41
# baseline (speedup 1.0000x reference)
"""MoChA (monotonic chunkwise attention, parallel mode) Trainium2 Bass kernel.

Problem: nn_MoChA_88596585382101 (sparse_attention, 8 cores, ridge regime).

Math (per batch row b):
    k    = key_enc @ wk_w.T + wk_b          [K, A]
    q    = query @ wq_w.T                   [A]
    v_w  = (v_g * v_v / ||v_v||)            [A]
    e    = tanh(k + q) . v_w + r            [K]
    p    = sigmoid(e + noise)               [K]
    # reference: cp = exp(cumsum([1, log(clip(1-p,EPS,1))[:-1]]))
    #            aw = p * cp * cumsum(onehot0 / clip(cp,EPS,1))
    # the third factor is identically 1 (cp[0]=e^1 clips to 1.0), so
    #            aw[j] = e^1 * p[j] * prod_{i<j} clip(1-p[i], EPS, 1)
    cv   = aw @ value                       [DV]

Sharding: data-parallel over batch B=16 across 8 cores (2 rows/core),
params replicated; full inputs sharded host-side, outputs concatenated.

Per-core pipeline (per 512-seq tile, fully unrolled under Tile):
  - key_enc tiles PE-transposed ([seq,dk]->[dk,seq], fp32, 2cyc/row) via a
    PSUM stage, DVE-copied to SBUF as float32r,
  - mm1: k^T[a,seq] = wk_w^T-chunks (stationary, f32r) x keyT (moving,
    N=512, f32r streams 1 cyc/row), 4 dk-chunks accumulated in PSUM,
  - tanh(k^T + qb[a]) fused on ACT straight out of PSUM (per-partition
    bias = wq_w @ query + wk_b, computed once on DVE via accum_out),
  - mm2: e[1,seq] = v_w (stationary) x tanh (moving f32r), 4 a-chunks
    accumulated; deferred one a-chunk behind mm1 so the PE never waits,
  - scan, per half-row (2048 seq) so it overlaps later projections:
    e repartitioned [1,2048]->[16,128] with two PE-transpose hops, then
    z = e + r + noise, p = sigmoid(z), q = max(1-p, EPS) on ACT/DVE, and
    the cumulative product of q via DVE tensor_tensor_scan (exclusive via
    a 129-wide buffer with a leading 1); cross-partition carries from a
    [16]->[1,16] PE transpose + a second scan seeded with e^1 (halves
    chained through the carry buffer tail),
  - mm3: cv[1,512] += aw columns (stationary f32r) x value chunks
    (moving f32r, cast on SWDGE load), 32 chunks accumulated in PSUM.

DMA: keys + weights on the two HWDGE queues (sync + scalar), values /
small params on the SWDGE queue, key issue runs 4 tiles ahead and value
issue lags 4 tiles to keep early bandwidth for keys.

float32r = fp32 bits with ~12-bit-mantissa rounding on the PE streaming
path (measured); end-to-end vs the fp32 reference: aw rel-err ~4e-6,
cv rel-err ~1.4e-4 (mm3 operand rounding).

Cost-model timeline: ~134.7 us/core (HBM floor ~100 us, PE busy ~116 us).
"""

import numpy as np
from contextlib import ExitStack

import concourse.bacc as bacc
import concourse.mybir as mybir
import concourse.tile as tile
import concourse.masks as masks
from concourse.bass_utils import run_bass_kernel_spmd

dt = mybir.dt
Act = mybir.ActivationFunctionType
Alu = mybir.AluOpType

N_CORES = 8
B, KMAX, DK, A, DV = 16, 4096, 512, 512, 512
BL = B // N_CORES          # batch rows per core
NT = KMAX // 512           # seq tiles of 512 per row
EPS = 1e-10
E1 = float(np.exp(1.0))

_NC = None


def _build():
    nc = bacc.Bacc("TRN2", target_bir_lowering=False, debug=False,
                   num_devices=N_CORES)
    key_ap = nc.dram_tensor("key_enc", [BL, KMAX, DK], dt.float32,
                            kind="ExternalInput").ap()
    val_ap = nc.dram_tensor("value", [BL, KMAX, DV], dt.float32,
                            kind="ExternalInput").ap()
    qry_ap = nc.dram_tensor("query", [BL, 1, DK], dt.float32,
                            kind="ExternalInput").ap()
    noi_ap = nc.dram_tensor("noise", [BL, KMAX], dt.float32,
                            kind="ExternalInput").ap()
    wkw_ap = nc.dram_tensor("wk_w", [A, DK], dt.float32,
                            kind="ExternalInput").ap()
    wkb_ap = nc.dram_tensor("wk_b", [A], dt.float32,
                            kind="ExternalInput").ap()
    wqw_ap = nc.dram_tensor("wq_w", [A, DK], dt.float32,
                            kind="ExternalInput").ap()
    vv_ap = nc.dram_tensor("v_v", [1, A], dt.float32,
                           kind="ExternalInput").ap()
    vg_ap = nc.dram_tensor("v_g", [1], dt.float32,
                           kind="ExternalInput").ap()
    r_ap = nc.dram_tensor("r", [1], dt.float32, kind="ExternalInput").ap()
    cv_ap = nc.dram_tensor("cv", [BL, 1, DV], dt.float32,
                           kind="ExternalOutput").ap()
    aw_ap = nc.dram_tensor("aw", [BL, KMAX, 1], dt.float32,
                           kind="ExternalOutput").ap()

    with tile.TileContext(nc) as tc:
        with ExitStack() as ctx:
            pre = ctx.enter_context(tc.tile_pool(name="pre", bufs=1))
            keyp = ctx.enter_context(tc.tile_pool(name="keyp", bufs=4))
            ktp = ctx.enter_context(tc.tile_pool(name="ktp", bufs=3))
            thp = ctx.enter_context(tc.tile_pool(name="thp", bufs=2))
            valp = ctx.enter_context(tc.tile_pool(name="valp", bufs=29))
            rowp = ctx.enter_context(tc.tile_pool(name="rowp", bufs=2))
            ps_mm1 = ctx.enter_context(
                tc.tile_pool(name="ps_mm1", bufs=3, space="PSUM"))
            ps_stage = ctx.enter_context(
                tc.tile_pool(name="ps_stage", bufs=3, space="PSUM"))
            ps_cv = ctx.enter_context(
                tc.tile_pool(name="ps_cv", bufs=1, space="PSUM"))
            ps_sm = ctx.enter_context(
                tc.tile_pool(name="ps_sm", bufs=1, space="PSUM"))

            ident = pre.tile([128, 128], dt.float32)
            masks.make_identity(nc, ident[:])
            # PE warm-up: keep the array busy (pstate/HAM ramp) while the
            # first wk/key DMAs are in flight; results are never read.
            warm_ps = ps_sm.tile([128, 128], dt.float32, tag="sm",
                                 name="warm_ps")
            for _ in range(10):
                nc.tensor.transpose(warm_ps[:], ident[:], ident[:])

            # ---- small parameter loads (SWDGE; v_v first for the norm) ----
            vv_sb = pre.tile([1, A], dt.float32)
            nc.gpsimd.dma_start(vv_sb[:], vv_ap)
            vg_sb = pre.tile([1, 1], dt.float32)
            nc.gpsimd.dma_start(vg_sb[:], vg_ap.unsqueeze(0))
            r_sb = pre.tile([1, 1], dt.float32)
            nc.gpsimd.dma_start(r_sb[:], r_ap.unsqueeze(0))
            qry_row = pre.tile([1, BL * 512], dt.float32)
            nc.gpsimd.dma_start(
                qry_row[:], qry_ap.rearrange("b one d -> one (b d)"))
            wkb_sb = pre.tile([128, 4], dt.float32)
            nc.gpsimd.dma_start(wkb_sb[:], wkb_ap.rearrange("(c p) -> p c", p=128))
            # ---- v_w scalar chain (early: ACT sqrt before the tanh set) ----
            sq = pre.tile([1, A], dt.float32)
            nc.vector.tensor_mul(sq[:], vv_sb[:], vv_sb[:])
            ss = pre.tile([1, 1], dt.float32)
            nc.vector.reduce_sum(ss[:], sq[:], axis=mybir.AxisListType.X)
            s0 = pre.tile([1, 1], dt.float32)
            nc.scalar.sqrt(s0[:], ss[:])
            rt0 = pre.tile([1, 1], dt.float32)
            nc.vector.reciprocal(rt0[:], s0[:])
            s1 = pre.tile([1, 1], dt.float32)
            nc.vector.tensor_mul(s1[:], ss[:], rt0[:])
            nc.vector.tensor_add(s1[:], s1[:], s0[:])
            nc.vector.tensor_scalar_mul(s1[:], s1[:], 0.5)
            inv = pre.tile([1, 1], dt.float32)
            nc.vector.reciprocal(inv[:], s1[:])
            scl = pre.tile([1, 1], dt.float32)
            nc.vector.tensor_mul(scl[:], inv[:], vg_sb[:])
            vw_row = pre.tile([1, A], dt.float32)
            nc.vector.tensor_scalar_mul(vw_row[:], vv_sb[:], scl[0:1, 0:1])
            # ---- wk_w^T and wq_w^T (PE transpose of 128x128 blocks) ----
            def load_transposed(w_ap, out_dtype, nm):
                chunks = []
                for c in range(4):
                    t_ = keyp.tile([128, 512], dt.float32, tag=f"key{c}",
                                   name=f"{nm}_ld{c}")
                    eng = nc.sync if c < 2 else nc.scalar
                    eng.dma_start(t_[:], w_ap[c * 128:(c + 1) * 128, :])
                    chunks.append(t_)
                outs = [pre.tile([128, 512], out_dtype, name=f"{nm}T{d}")
                        for d in range(4)]
                for c in (0, 2, 1, 3):  # DMA arrival order (2 queues)
                    stg = ps_stage.tile([128, 512], dt.float32, tag="stg",
                                        name=f"{nm}_cstg{c}")
                    for d in range(4):
                        nc.tensor.transpose(
                            stg[:, d * 128:(d + 1) * 128],
                            chunks[c][:, d * 128:(d + 1) * 128], ident[:])
                    for d in range(4):
                        nc.vector.tensor_copy(
                            outs[d][:, c * 128:(c + 1) * 128],
                            stg[:, d * 128:(d + 1) * 128])
                return outs

            wkT = load_transposed(wkw_ap, dt.float32r, "wk")
            # wq_w natural chunks (for the DVE qb reduction) — sync queue
            # first so query projection can start immediately
            wq_sb = []
            for c in range(4):
                t_ = pre.tile([128, 512], dt.float32, name=f"wq_ld{c}")
                nc.sync.dma_start(t_[:], wqw_ap[c * 128:(c + 1) * 128, :])
                wq_sb.append(t_)



            vw_sb = pre.tile([128, 4], dt.float32r)
            for c in range(4):
                tp = ps_sm.tile([128, 32], dt.float32, tag="sm",
                                name=f"vwT{c}")
                nc.tensor.transpose(tp[:, 0:1],
                                    vw_row[0:1, c * 128:(c + 1) * 128],
                                    ident[0:1, 0:1])
                nc.vector.tensor_copy(vw_sb[:, c:c + 1], tp[:, 0:1])

            # ---- qb[b] = wq_w @ query[b] + wk_b ----
            # query broadcast along partitions (ones outer product on PE),
            # then one fused multiply+row-sum (accum_out) per (b, a-chunk).
            ones_col = pre.tile([1, 128], dt.float32)
            nc.vector.memset(ones_col[:], 1.0)
            qb_sb = pre.tile([128, BL * 4], dt.float32)
            qtmp = pre.tile([128, 512], dt.float32)
            qbcast = pre.tile([128, 512], dt.float32)
            def qb_chain(b):
                qbc_ps = ps_stage.tile([128, 512], dt.float32, tag="stg",
                                       name=f"qbc_ps{b}")
                nc.tensor.matmul(qbc_ps[:], ones_col[:],
                                 qry_row[0:1, b * 512:(b + 1) * 512],
                                 start=True, stop=True)
                nc.vector.tensor_copy(qbcast[:], qbc_ps[:])
                for c in range(4):
                    nc.vector.scalar_tensor_tensor(
                        qtmp[:], wq_sb[c][:], 0.0, qbcast[:],
                        op0=Alu.add, op1=Alu.mult,
                        accum_out=qb_sb[:, 4 * b + c:4 * b + c + 1])
                for c in range(4):
                    nc.vector.tensor_add(qb_sb[:, 4 * b + c:4 * b + c + 1],
                                         qb_sb[:, 4 * b + c:4 * b + c + 1],
                                         wkb_sb[:, c:c + 1])
            qb_chain(0)


            erow = {}
            noise32s = {}
            pbufs = {}
            cbufs = {}
            for b in range(BL):
                for h in range(2):
                    pbufs[(b, h)] = rowp.tile([16, 129], dt.float32,
                                              tag="pbuf", bufs=4,
                                              name=f"pbuf{b}_{h}")
                    nc.vector.memset(pbufs[(b, h)][:, 0:1], 1.0)
                    cbufs[(b, h)] = rowp.tile([1, 17], dt.float32,
                                              tag="cbuf", bufs=4,
                                              name=f"cbuf{b}_{h}")
                    if h == 0:
                        nc.vector.memset(cbufs[(b, h)][0:1, 0:1], E1)
            vals = {b: [] for b in range(BL)}
            for b in range(BL):
                erow[b] = rowp.tile([1, KMAX], dt.float32, tag="erow",
                                    name=f"erow{b}")
                for h in range(2):
                    noise32s[(b, h)] = rowp.tile(
                        [16, 128], dt.float32, tag="noi", bufs=4,
                        name=f"noi{b}_{h}")
                    nc.gpsimd.dma_start(
                        noise32s[(b, h)][:],
                        noi_ap[b:b + 1, h * 2048:(h + 1) * 2048].rearrange(
                            "one (q f) -> q f", q=16))
            pending = {}
            pend_mm2 = [None]

            def flush_mm2():
                if pend_mm2[0] is not None:
                    fn = pend_mm2[0]
                    pend_mm2[0] = None
                    fn()

            def issue_key_dmas(b, t):
                base = t * 512
                ksb = []
                for j in range(4):
                    kt = keyp.tile([128, 512], dt.float32, tag=f"key{j}",
                                   name=f"key{b}_{t}_{j}")
                    nc.sync.dma_start(
                        kt[:],
                        key_ap[b, base + j * 128:base + (j + 1) * 128, :])
                    ksb.append(kt)
                pending[(b, t)] = ksb

            def issue_val_dmas(b, t):
                for cc in range(4):
                    idx = t * 4 + cc
                    vt = valp.tile([128, 512], dt.float32r, tag="val",
                                   name=f"val{b}_{idx}")
                    nc.gpsimd.dma_start(
                        vt[:], val_ap[b, idx * 128:(idx + 1) * 128, :])
                    vals[b].append(vt)

            tiles = [(0, t) for t in range(NT)] + [(1, 0), (1, 1)] \
                + [(1, t) for t in range(2, NT)]
            LOOKAHEAD = 4
            VAL_LAG = 4
            for i in range(LOOKAHEAD):
                issue_key_dmas(*tiles[i])

            # ---- r broadcast to [32,1]; ones tiles ----
            ones_r = pre.tile([1, 32], dt.float32)
            nc.vector.memset(ones_r[:], 1.0)
            r32_ps = ps_sm.tile([128, 32], dt.float32, tag="sm", name="r32ps")
            nc.tensor.matmul(r32_ps[0:32, 0:1], ones_r[:], r_sb[:],
                             start=True, stop=True)
            r32_sb = pre.tile([32, 1], dt.float32)
            nc.vector.tensor_copy(r32_sb[:], r32_ps[0:32, 0:1])
            ones32 = pre.tile([32, 128], dt.float32)
            nc.vector.memset(ones32[:], 1.0)



            def proj_tile(b, t):
                """One 512-seq tile of the projection phase for row b."""
                base = t * 512
                ksb = pending.pop((b, t))
                kT = []
                for d in range(4):
                    stg = ps_stage.tile([128, 512], dt.float32, tag="stg",
                                        name=f"stg{b}_{t}_{d}")
                    for j in range(4):
                        nc.tensor.transpose(
                            stg[:, j * 128:(j + 1) * 128],
                            ksb[j][:, d * 128:(d + 1) * 128], ident[:])
                    kt_ = ktp.tile([128, 512], dt.float32r, tag=f"kT{d}",
                                   name=f"kT{b}_{t}_{d}")
                    nc.vector.tensor_copy(kt_[:], stg[:])
                    kT.append(kt_)
                if len(kT) == 4:
                    flush_mm2()
                e_ps = ps_sm.tile([1, 512], dt.float32, tag="sm",
                                 name=f"e_ps{b}_{t}")
                ths = []
                for c in range(4):
                    kp = ps_mm1.tile([128, 512], dt.float32, tag="mm1",
                                     name=f"k_ps{b}_{t}_{c}")
                    for d in range(4):
                        nc.tensor.matmul(kp[:],
                                         wkT[d][:, c * 128:(c + 1) * 128],
                                         kT[d][:],
                                         start=(d == 0), stop=(d == 3))
                    th = thp.tile([128, 512], dt.float32r, tag=f"th{c}",
                                  name=f"th{b}_{t}_{c}")
                    nc.scalar.activation(th[:], kp[:], Act.Tanh,
                                         bias=qb_sb[:, 4 * b + c:4 * b + c + 1])
                    ths.append(th)
                    if c >= 1:  # defer mm2 one a-chunk so PE never waits ACT
                        cm = c - 1
                        nc.tensor.matmul(e_ps[:], vw_sb[:, cm:cm + 1],
                                         ths[cm][:],
                                         start=(cm == 0), stop=False)

                def _mm2_tail(e_ps=e_ps, th3=ths[3], b=b, base=base):
                    # last a-chunk + e copy, emitted after the NEXT tile's
                    # transposes so the PE never waits on tanh(c3)
                    nc.tensor.matmul(e_ps[:], vw_sb[:, 3:4], th3[:],
                                     start=False, stop=True)
                    nc.scalar.copy(erow[b][0:1, base:base + 512], e_ps[:])
                pend_mm2[0] = _mm2_tail

            cvps = {}

            def scan_cv_half(b, h):
                """Scan chain + cv contraction for row b, half h (2048 seq)."""
                ecols_ps = ps_sm.tile([128, 16], dt.float32, tag="sm",
                                      name=f"ecols_ps{b}_{h}")
                for q in range(16):
                    qq = h * 16 + q
                    nc.tensor.transpose(ecols_ps[:, q:q + 1],
                                        erow[b][0:1, qq * 128:(qq + 1) * 128],
                                        ident[0:1, 0:1])
                ecols = rowp.tile([128, 16], dt.float32, tag="ecols",
                                  name=f"ecols{b}_{h}")
                nc.vector.tensor_copy(ecols[:], ecols_ps[:])
                e32_ps = ps_sm.tile([16, 128], dt.float32, tag="sm",
                                    name=f"e32_ps{b}_{h}")
                nc.tensor.transpose(e32_ps[:], ecols[:], ident[:])
                noise32 = noise32s[(b, h)]
                z32 = rowp.tile([16, 128], dt.float32, tag="z32",
                                name=f"z32_{b}_{h}")
                nc.vector.scalar_tensor_tensor(
                    z32[:], e32_ps[:], r32_sb[0:16, :], noise32[:],
                    op0=Alu.add, op1=Alu.add)
                p32 = rowp.tile([16, 128], dt.float32, tag="p32",
                                name=f"p32_{b}_{h}")
                nc.scalar.activation(p32[:], z32[:], Act.Sigmoid)
                qc = rowp.tile([16, 128], dt.float32, tag="qc",
                               name=f"qc{b}_{h}")
                nc.vector.scalar_tensor_tensor(
                    qc[:], p32[:], -1.0, ones32[0:16, :],
                    op0=Alu.mult, op1=Alu.add)
                nc.vector.tensor_scalar_max(qc[:], qc[:], EPS)
                pbuf = pbufs[(b, h)]
                nc.vector.tensor_tensor_scan(
                    pbuf[:, 1:129], qc[:], ones32[0:16, :], 1.0,
                    op0=Alu.mult, op1=Alu.mult)
                # cross-partition carries; chain across halves via cbuf[16]
                totT_ps = ps_sm.tile([1, 16], dt.float32, tag="sm",
                                     name=f"totT{b}_{h}")
                nc.tensor.transpose(totT_ps[:], pbuf[:, 128:129],
                                    ident[0:16, 0:16])
                cbuf = cbufs[(b, h)]
                init = E1 if h == 0 else cbufs[(b, 0)][0:1, 16:17]
                if h == 1:
                    nc.vector.tensor_copy(cbuf[0:1, 0:1],
                                          cbufs[(b, 0)][0:1, 16:17])
                nc.vector.tensor_tensor_scan(
                    cbuf[0:1, 1:17], totT_ps[:], ones_r[0:1, 0:16], init,
                    op0=Alu.mult, op1=Alu.mult)
                carryT_ps = ps_sm.tile([16, 1], dt.float32, tag="sm",
                                       name=f"carryT{b}_{h}")
                nc.tensor.transpose(carryT_ps[:], cbuf[0:1, 0:16],
                                    ident[0:1, 0:1])
                carry32 = rowp.tile([16, 1], dt.float32, tag="carry",
                                    name=f"carry{b}_{h}")
                nc.vector.tensor_copy(carry32[:], carryT_ps[:])
                aw32 = rowp.tile([16, 128], dt.float32, tag="aw",
                                 name=f"aw{b}_{h}")
                nc.vector.scalar_tensor_tensor(
                    aw32[:], pbuf[:, 0:128], carry32[:], p32[:],
                    op0=Alu.mult, op1=Alu.mult)
                nc.sync.dma_start(
                    aw_ap[b, h * 2048:(h + 1) * 2048, 0].rearrange(
                        "(q f) -> q f", q=16), aw32[:])
                awT_ps = ps_sm.tile([128, 16], dt.float32, tag="sm",
                                    name=f"awT_ps{b}_{h}")
                nc.tensor.transpose(awT_ps[:], aw32[:], ident[0:16, 0:16])
                awT = rowp.tile([128, 16], dt.float32r, tag="awT",
                                name=f"awT{b}_{h}")
                nc.vector.tensor_copy(awT[:], awT_ps[:])
                if h == 0:
                    cvps[b] = ps_cv.tile([1, 512], dt.float32, tag="cv",
                                         name=f"cvps{b}")
                cvp = cvps[b]
                for q2 in range(16):
                    nc.tensor.matmul(cvp[:], awT[:, q2:q2 + 1],
                                     vals[b][h * 16 + q2][:],
                                     start=(h == 0 and q2 == 0),
                                     stop=(h == 1 and q2 == 15))
                if h == 1:
                    cvs = rowp.tile([1, 512], dt.float32, tag="cvs",
                                    name=f"cvs{b}")
                    nc.vector.tensor_copy(cvs[:], cvp[:])
                    nc.sync.dma_start(cv_ap[b:b + 1, 0, :], cvs[:])

            # row 0 projections; start row 1 before row 0's scan so the PE
            # has independent work while the scan chain runs.  DMA issue runs
            # LOOKAHEAD tiles ahead of compute so SDMA never starves the PE.
            di = LOOKAHEAD
            done = 0
            for b, t in tiles:
                proj_tile(b, t)
                if di < len(tiles):
                    issue_key_dmas(*tiles[di])
                    di += 1
                vi = done - VAL_LAG + LOOKAHEAD
                if 0 <= vi < len(tiles):
                    issue_val_dmas(*tiles[vi])
                done += 1
                if done == 2:
                    qb_chain(1)
                if done == 4:
                    flush_mm2()
                    scan_cv_half(0, 0)
                if done == NT + 2:
                    flush_mm2()
                    scan_cv_half(0, 1)
                if done == 15:
                    flush_mm2()
                    scan_cv_half(1, 0)
            for vi in range(len(tiles) - VAL_LAG + LOOKAHEAD, len(tiles)):
                if vi >= 0:
                    issue_val_dmas(*tiles[vi])
            flush_mm2()
            scan_cv_half(1, 1)

    nc.finalize()
    return nc


def _get_nc():
    global _NC
    if _NC is None:
        _NC = _build()
    return _NC


def _shard_inputs(inputs):
    f32 = lambda x: np.ascontiguousarray(np.asarray(x), dtype=np.float32)
    full = {k: f32(v) for k, v in inputs.items()}
    in_maps = []
    for g in range(N_CORES):
        sl = slice(g * BL, (g + 1) * BL)
        in_maps.append({
            "key_enc": full["key_enc"][sl],
            "value": full["value"][sl],
            "query": full["query"][sl],
            "noise": full["noise"][sl],
            "wk_w": full["wk_w"],
            "wk_b": full["wk_b"],
            "wq_w": full["wq_w"],
            "v_v": full["v_v"],
            "v_g": full["v_g"],
            "r": full["r"],
        })
    return in_maps


def run_sharded(inputs, **kw):
    """Run on 8 cores; returns (cv, aw) full outputs + BassKernelResults."""
    nc = _get_nc()
    res = run_bass_kernel_spmd(nc, _shard_inputs(inputs),
                               core_ids=list(range(N_CORES)), **kw)
    cv = np.concatenate([res.results[g]["cv"] for g in range(N_CORES)], axis=0)
    aw = np.concatenate([res.results[g]["aw"] for g in range(N_CORES)], axis=0)
    return cv, aw, res


def kernel(**inputs):
    cv, aw, _ = run_sharded(inputs)
    return cv, aw


# revision 43
# speedup vs baseline: 1.0026x; 1.0026x over previous
"""MoChA (monotonic chunkwise attention, parallel mode) Trainium2 Bass kernel.

Problem: nn_MoChA_88596585382101 (sparse_attention, 8 cores, ridge regime).

Math (per batch row b):
    k    = key_enc @ wk_w.T + wk_b          [K, A]
    q    = query @ wq_w.T                   [A]
    v_w  = (v_g * v_v / ||v_v||)            [A]
    e    = tanh(k + q) . v_w + r            [K]
    p    = sigmoid(e + noise)               [K]
    # reference: cp = exp(cumsum([1, log(clip(1-p,EPS,1))[:-1]]))
    #            aw = p * cp * cumsum(onehot0 / clip(cp,EPS,1))
    # the third factor is identically 1 (cp[0]=e^1 clips to 1.0), so
    #            aw[j] = e^1 * p[j] * prod_{i<j} clip(1-p[i], EPS, 1)
    cv   = aw @ value                       [DV]

Sharding: data-parallel over batch B=16 across 8 cores (2 rows/core),
params replicated; full inputs sharded host-side, outputs concatenated.

Per-core pipeline (per 512-seq tile, fully unrolled under Tile):
  - key_enc tiles PE-transposed ([seq,dk]->[dk,seq], fp32, 2cyc/row) via a
    PSUM stage, DVE-copied to SBUF as float32r,
  - mm1: k^T[a,seq] = wk_w^T-chunks (stationary, f32r) x keyT (moving,
    N=512, f32r streams 1 cyc/row), 4 dk-chunks accumulated in PSUM,
  - tanh(k^T + qb[a]) fused on ACT straight out of PSUM (per-partition
    bias = wq_w @ query + wk_b, computed once on DVE via accum_out),
  - mm2: e[1,seq] = v_w (stationary) x tanh (moving f32r), 4 a-chunks
    accumulated; deferred one a-chunk behind mm1 so the PE never waits,
  - scan, per half-row (2048 seq) so it overlaps later projections:
    e repartitioned [1,2048]->[16,128] with two PE-transpose hops, then
    z = e + r + noise, p = sigmoid(z), q = max(1-p, EPS) on ACT/DVE, and
    the cumulative product of q via DVE tensor_tensor_scan (exclusive via
    a 129-wide buffer with a leading 1); cross-partition carries from a
    [16]->[1,16] PE transpose + a second scan seeded with e^1 (halves
    chained through the carry buffer tail),
  - mm3: cv[1,512] += aw columns (stationary f32r) x value chunks
    (moving f32r, cast on SWDGE load), 32 chunks accumulated in PSUM.

DMA: keys + weights on the two HWDGE queues (sync + scalar), values /
small params on the SWDGE queue, key issue runs 4 tiles ahead and value
issue lags 4 tiles to keep early bandwidth for keys.

float32r = fp32 bits with ~12-bit-mantissa rounding on the PE streaming
path (measured); end-to-end vs the fp32 reference: aw rel-err ~4e-6,
cv rel-err ~1.4e-4 (mm3 operand rounding).

Cost-model timeline: ~134.7 us/core (HBM floor ~100 us, PE busy ~116 us).
"""

import numpy as np
from contextlib import ExitStack

import concourse.bacc as bacc
import concourse.mybir as mybir
import concourse.tile as tile
import concourse.masks as masks
from concourse.bass_utils import run_bass_kernel_spmd

dt = mybir.dt
Act = mybir.ActivationFunctionType
Alu = mybir.AluOpType

N_CORES = 8
B, KMAX, DK, A, DV = 16, 4096, 512, 512, 512
BL = B // N_CORES          # batch rows per core
NT = KMAX // 512           # seq tiles of 512 per row
EPS = 1e-10
E1 = float(np.exp(1.0))

_NC = None


def _build():
    nc = bacc.Bacc("TRN2", target_bir_lowering=False, debug=False,
                   num_devices=N_CORES)
    key_ap = nc.dram_tensor("key_enc", [BL, KMAX, DK], dt.float32,
                            kind="ExternalInput").ap()
    val_ap = nc.dram_tensor("value", [BL, KMAX, DV], dt.float32,
                            kind="ExternalInput").ap()
    qry_ap = nc.dram_tensor("query", [BL, 1, DK], dt.float32,
                            kind="ExternalInput").ap()
    noi_ap = nc.dram_tensor("noise", [BL, KMAX], dt.float32,
                            kind="ExternalInput").ap()
    wkw_ap = nc.dram_tensor("wk_w", [A, DK], dt.float32,
                            kind="ExternalInput").ap()
    wkb_ap = nc.dram_tensor("wk_b", [A], dt.float32,
                            kind="ExternalInput").ap()
    wqw_ap = nc.dram_tensor("wq_w", [A, DK], dt.float32,
                            kind="ExternalInput").ap()
    vv_ap = nc.dram_tensor("v_v", [1, A], dt.float32,
                           kind="ExternalInput").ap()
    vg_ap = nc.dram_tensor("v_g", [1], dt.float32,
                           kind="ExternalInput").ap()
    r_ap = nc.dram_tensor("r", [1], dt.float32, kind="ExternalInput").ap()
    cv_ap = nc.dram_tensor("cv", [BL, 1, DV], dt.float32,
                           kind="ExternalOutput").ap()
    aw_ap = nc.dram_tensor("aw", [BL, KMAX, 1], dt.float32,
                           kind="ExternalOutput").ap()

    with tile.TileContext(nc) as tc:
        with ExitStack() as ctx:
            pre = ctx.enter_context(tc.tile_pool(name="pre", bufs=1))
            keyp = ctx.enter_context(tc.tile_pool(name="keyp", bufs=4))
            ktp = ctx.enter_context(tc.tile_pool(name="ktp", bufs=3))
            thp = ctx.enter_context(tc.tile_pool(name="thp", bufs=2))
            valp = ctx.enter_context(tc.tile_pool(name="valp", bufs=29))
            rowp = ctx.enter_context(tc.tile_pool(name="rowp", bufs=2))
            ps_mm1 = ctx.enter_context(
                tc.tile_pool(name="ps_mm1", bufs=3, space="PSUM"))
            ps_stage = ctx.enter_context(
                tc.tile_pool(name="ps_stage", bufs=3, space="PSUM"))
            ps_cv = ctx.enter_context(
                tc.tile_pool(name="ps_cv", bufs=1, space="PSUM"))
            ps_sm = ctx.enter_context(
                tc.tile_pool(name="ps_sm", bufs=1, space="PSUM"))

            ident = pre.tile([128, 128], dt.float32)
            masks.make_identity(nc, ident[:])
            # PE warm-up: keep the array busy (pstate/HAM ramp) while the
            # first wk/key DMAs are in flight; results are never read.
            warm_ps = ps_sm.tile([128, 128], dt.float32, tag="sm",
                                 name="warm_ps")
            for _ in range(10):
                nc.tensor.transpose(warm_ps[:], ident[:], ident[:])

            # ---- small parameter loads (SWDGE; v_v first for the norm) ----
            vv_sb = pre.tile([1, A], dt.float32)
            nc.gpsimd.dma_start(vv_sb[:], vv_ap)
            vg_sb = pre.tile([1, 1], dt.float32)
            nc.gpsimd.dma_start(vg_sb[:], vg_ap.unsqueeze(0))
            r_sb = pre.tile([1, 1], dt.float32)
            nc.gpsimd.dma_start(r_sb[:], r_ap.unsqueeze(0))
            qry_row = pre.tile([1, BL * 512], dt.float32)
            nc.gpsimd.dma_start(
                qry_row[:], qry_ap.rearrange("b one d -> one (b d)"))
            wkb_sb = pre.tile([128, 4], dt.float32)
            nc.gpsimd.dma_start(wkb_sb[:], wkb_ap.rearrange("(c p) -> p c", p=128))
            # ---- v_w scalar chain (early: ACT sqrt before the tanh set) ----
            sq = pre.tile([1, A], dt.float32)
            nc.vector.tensor_mul(sq[:], vv_sb[:], vv_sb[:])
            ss = pre.tile([1, 1], dt.float32)
            nc.vector.reduce_sum(ss[:], sq[:], axis=mybir.AxisListType.X)
            s0 = pre.tile([1, 1], dt.float32)
            nc.scalar.sqrt(s0[:], ss[:])
            rt0 = pre.tile([1, 1], dt.float32)
            nc.vector.reciprocal(rt0[:], s0[:])
            s1 = pre.tile([1, 1], dt.float32)
            nc.vector.tensor_mul(s1[:], ss[:], rt0[:])
            nc.vector.tensor_add(s1[:], s1[:], s0[:])
            nc.vector.tensor_scalar_mul(s1[:], s1[:], 0.5)
            inv = pre.tile([1, 1], dt.float32)
            nc.vector.reciprocal(inv[:], s1[:])
            scl = pre.tile([1, 1], dt.float32)
            nc.vector.tensor_mul(scl[:], inv[:], vg_sb[:])
            vw_row = pre.tile([1, A], dt.float32)
            nc.vector.tensor_scalar_mul(vw_row[:], vv_sb[:], scl[0:1, 0:1])
            # ---- wk_w^T and wq_w^T (PE transpose of 128x128 blocks) ----
            def load_transposed(w_ap, out_dtype, nm):
                chunks = []
                for c in range(4):
                    t_ = keyp.tile([128, 512], dt.float32, tag=f"key{c}",
                                   name=f"{nm}_ld{c}")
                    eng = nc.sync if c < 2 else nc.scalar
                    eng.dma_start(t_[:], w_ap[c * 128:(c + 1) * 128, :])
                    chunks.append(t_)
                outs = [pre.tile([128, 512], out_dtype, name=f"{nm}T{d}")
                        for d in range(4)]
                for c in (0, 2, 1, 3):  # DMA arrival order (2 queues)
                    stg = ps_stage.tile([128, 512], dt.float32, tag="stg",
                                        name=f"{nm}_cstg{c}")
                    for d in range(4):
                        nc.tensor.transpose(
                            stg[:, d * 128:(d + 1) * 128],
                            chunks[c][:, d * 128:(d + 1) * 128], ident[:])
                    for d in range(4):
                        nc.vector.tensor_copy(
                            outs[d][:, c * 128:(c + 1) * 128],
                            stg[:, d * 128:(d + 1) * 128])
                return outs

            wkT = load_transposed(wkw_ap, dt.float32r, "wk")



            vw_sb = pre.tile([128, 4], dt.float32r)
            for c in range(4):
                tp = ps_sm.tile([128, 32], dt.float32, tag="sm",
                                name=f"vwT{c}")
                nc.tensor.transpose(tp[:, 0:1],
                                    vw_row[0:1, c * 128:(c + 1) * 128],
                                    ident[0:1, 0:1])
                nc.vector.tensor_copy(vw_sb[:, c:c + 1], tp[:, 0:1])

            # ---- qb[b] = wq_w @ query[b] + wk_b ----
            # query broadcast along partitions (ones outer product on PE),
            # then one fused multiply+row-sum (accum_out) per (b, a-chunk).
            ones_col = pre.tile([1, 128], dt.float32)
            nc.vector.memset(ones_col[:], 1.0)
            qb_sb = pre.tile([128, BL * 4], dt.float32)
            qtmp = pre.tile([128, 512], dt.float32)
            qbcast = pre.tile([128, 512], dt.float32)
            def qb_chain(b):
                qbc_ps = ps_stage.tile([128, 512], dt.float32, tag="stg",
                                       name=f"qbc_ps{b}")
                nc.tensor.matmul(qbc_ps[:], ones_col[:],
                                 qry_row[0:1, b * 512:(b + 1) * 512],
                                 start=True, stop=True)
                nc.vector.tensor_copy(qbcast[:], qbc_ps[:])
                for c in range(4):
                    nc.vector.scalar_tensor_tensor(
                        qtmp[:], wq_sb[c][:], 0.0, qbcast[:],
                        op0=Alu.add, op1=Alu.mult,
                        accum_out=qb_sb[:, 4 * b + c:4 * b + c + 1])
                for c in range(4):
                    nc.vector.tensor_add(qb_sb[:, 4 * b + c:4 * b + c + 1],
                                         qb_sb[:, 4 * b + c:4 * b + c + 1],
                                         wkb_sb[:, c:c + 1])



            erow = {}
            noise32s = {}
            pbufs = {}
            cbufs = {}
            for b in range(BL):
                for h in range(2):
                    pbufs[(b, h)] = rowp.tile([16, 129], dt.float32,
                                              tag="pbuf", bufs=4,
                                              name=f"pbuf{b}_{h}")
                    nc.vector.memset(pbufs[(b, h)][:, 0:1], 1.0)
                    cbufs[(b, h)] = rowp.tile([1, 17], dt.float32,
                                              tag="cbuf", bufs=4,
                                              name=f"cbuf{b}_{h}")
                    if h == 0:
                        nc.vector.memset(cbufs[(b, h)][0:1, 0:1], E1)
            vals = {b: [] for b in range(BL)}
            for b in range(BL):
                erow[b] = rowp.tile([1, KMAX], dt.float32, tag="erow",
                                    name=f"erow{b}")
                for h in range(2):
                    noise32s[(b, h)] = rowp.tile(
                        [16, 128], dt.float32, tag="noi", bufs=4,
                        name=f"noi{b}_{h}")
                    nc.gpsimd.dma_start(
                        noise32s[(b, h)][:],
                        noi_ap[b:b + 1, h * 2048:(h + 1) * 2048].rearrange(
                            "one (q f) -> q f", q=16))
            pending = {}
            pend_mm2 = [None]

            def flush_mm2():
                if pend_mm2[0] is not None:
                    fn = pend_mm2[0]
                    pend_mm2[0] = None
                    fn()

            def issue_key_dmas(b, t):
                base = t * 512
                ksb = []
                for j in range(4):
                    kt = keyp.tile([128, 512], dt.float32, tag=f"key{j}",
                                   name=f"key{b}_{t}_{j}")
                    nc.sync.dma_start(
                        kt[:],
                        key_ap[b, base + j * 128:base + (j + 1) * 128, :])
                    ksb.append(kt)
                pending[(b, t)] = ksb

            def issue_val_dmas(b, t):
                for cc in range(4):
                    idx = t * 4 + cc
                    vt = valp.tile([128, 512], dt.float32r, tag="val",
                                   name=f"val{b}_{idx}")
                    nc.gpsimd.dma_start(
                        vt[:], val_ap[b, idx * 128:(idx + 1) * 128, :])
                    vals[b].append(vt)

            tiles = [(0, t) for t in range(NT)] + [(1, 0), (1, 1)] \
                + [(1, t) for t in range(2, NT)]
            LOOKAHEAD = 4
            VAL_LAG = 4
            issue_key_dmas(*tiles[0])
            # wq_w loads sit behind tile-0's keys on the sync queue: they
            # only feed the qb chain, which isn't needed until tanh(t0)
            wq_sb = []
            for c in range(4):
                t_ = pre.tile([128, 512], dt.float32, name=f"wq_ld{c}")
                nc.sync.dma_start(t_[:], wqw_ap[c * 128:(c + 1) * 128, :])
                wq_sb.append(t_)
            for i in range(1, LOOKAHEAD):
                issue_key_dmas(*tiles[i])
            qb_chain(0)

            # ---- r broadcast to [32,1]; ones tiles ----
            ones_r = pre.tile([1, 32], dt.float32)
            nc.vector.memset(ones_r[:], 1.0)
            r32_ps = ps_sm.tile([128, 32], dt.float32, tag="sm", name="r32ps")
            nc.tensor.matmul(r32_ps[0:32, 0:1], ones_r[:], r_sb[:],
                             start=True, stop=True)
            r32_sb = pre.tile([32, 1], dt.float32)
            nc.vector.tensor_copy(r32_sb[:], r32_ps[0:32, 0:1])
            ones32 = pre.tile([32, 128], dt.float32)
            nc.vector.memset(ones32[:], 1.0)



            def proj_tile(b, t):
                """One 512-seq tile of the projection phase for row b."""
                base = t * 512
                ksb = pending.pop((b, t))
                kT = []
                for d in range(4):
                    stg = ps_stage.tile([128, 512], dt.float32, tag="stg",
                                        name=f"stg{b}_{t}_{d}")
                    for j in range(4):
                        nc.tensor.transpose(
                            stg[:, j * 128:(j + 1) * 128],
                            ksb[j][:, d * 128:(d + 1) * 128], ident[:])
                    kt_ = ktp.tile([128, 512], dt.float32r, tag=f"kT{d}",
                                   name=f"kT{b}_{t}_{d}")
                    nc.vector.tensor_copy(kt_[:], stg[:])
                    kT.append(kt_)
                if len(kT) == 4:
                    flush_mm2()
                e_ps = ps_sm.tile([1, 512], dt.float32, tag="sm",
                                 name=f"e_ps{b}_{t}")
                ths = []
                for c in range(4):
                    kp = ps_mm1.tile([128, 512], dt.float32, tag="mm1",
                                     name=f"k_ps{b}_{t}_{c}")
                    for d in range(4):
                        nc.tensor.matmul(kp[:],
                                         wkT[d][:, c * 128:(c + 1) * 128],
                                         kT[d][:],
                                         start=(d == 0), stop=(d == 3))
                    th = thp.tile([128, 512], dt.float32r, tag=f"th{c}",
                                  name=f"th{b}_{t}_{c}")
                    nc.scalar.activation(th[:], kp[:], Act.Tanh,
                                         bias=qb_sb[:, 4 * b + c:4 * b + c + 1])
                    ths.append(th)
                    if c >= 1:  # defer mm2 one a-chunk so PE never waits ACT
                        cm = c - 1
                        nc.tensor.matmul(e_ps[:], vw_sb[:, cm:cm + 1],
                                         ths[cm][:],
                                         start=(cm == 0), stop=False)

                def _mm2_tail(e_ps=e_ps, th3=ths[3], b=b, base=base):
                    # last a-chunk + e copy, emitted after the NEXT tile's
                    # transposes so the PE never waits on tanh(c3)
                    nc.tensor.matmul(e_ps[:], vw_sb[:, 3:4], th3[:],
                                     start=False, stop=True)
                    nc.scalar.copy(erow[b][0:1, base:base + 512], e_ps[:])
                pend_mm2[0] = _mm2_tail

            cvps = {}

            def scan_cv_half(b, h):
                """Scan chain + cv contraction for row b, half h (2048 seq)."""
                ecols_ps = ps_sm.tile([128, 16], dt.float32, tag="sm",
                                      name=f"ecols_ps{b}_{h}")
                for q in range(16):
                    qq = h * 16 + q
                    nc.tensor.transpose(ecols_ps[:, q:q + 1],
                                        erow[b][0:1, qq * 128:(qq + 1) * 128],
                                        ident[0:1, 0:1])
                ecols = rowp.tile([128, 16], dt.float32, tag="ecols",
                                  name=f"ecols{b}_{h}")
                nc.vector.tensor_copy(ecols[:], ecols_ps[:])
                e32_ps = ps_sm.tile([16, 128], dt.float32, tag="sm",
                                    name=f"e32_ps{b}_{h}")
                nc.tensor.transpose(e32_ps[:], ecols[:], ident[:])
                noise32 = noise32s[(b, h)]
                z32 = rowp.tile([16, 128], dt.float32, tag="z32",
                                name=f"z32_{b}_{h}")
                nc.vector.scalar_tensor_tensor(
                    z32[:], e32_ps[:], r32_sb[0:16, :], noise32[:],
                    op0=Alu.add, op1=Alu.add)
                p32 = rowp.tile([16, 128], dt.float32, tag="p32",
                                name=f"p32_{b}_{h}")
                nc.scalar.activation(p32[:], z32[:], Act.Sigmoid)
                qc = rowp.tile([16, 128], dt.float32, tag="qc",
                               name=f"qc{b}_{h}")
                nc.vector.scalar_tensor_tensor(
                    qc[:], p32[:], -1.0, ones32[0:16, :],
                    op0=Alu.mult, op1=Alu.add)
                nc.vector.tensor_scalar_max(qc[:], qc[:], EPS)
                pbuf = pbufs[(b, h)]
                nc.vector.tensor_tensor_scan(
                    pbuf[:, 1:129], qc[:], ones32[0:16, :], 1.0,
                    op0=Alu.mult, op1=Alu.mult)
                # cross-partition carries; chain across halves via cbuf[16]
                totT_ps = ps_sm.tile([1, 16], dt.float32, tag="sm",
                                     name=f"totT{b}_{h}")
                nc.tensor.transpose(totT_ps[:], pbuf[:, 128:129],
                                    ident[0:16, 0:16])
                cbuf = cbufs[(b, h)]
                init = E1 if h == 0 else cbufs[(b, 0)][0:1, 16:17]
                if h == 1:
                    nc.vector.tensor_copy(cbuf[0:1, 0:1],
                                          cbufs[(b, 0)][0:1, 16:17])
                nc.vector.tensor_tensor_scan(
                    cbuf[0:1, 1:17], totT_ps[:], ones_r[0:1, 0:16], init,
                    op0=Alu.mult, op1=Alu.mult)
                carryT_ps = ps_sm.tile([16, 1], dt.float32, tag="sm",
                                       name=f"carryT{b}_{h}")
                nc.tensor.transpose(carryT_ps[:], cbuf[0:1, 0:16],
                                    ident[0:1, 0:1])
                carry32 = rowp.tile([16, 1], dt.float32, tag="carry",
                                    name=f"carry{b}_{h}")
                nc.vector.tensor_copy(carry32[:], carryT_ps[:])
                aw32 = rowp.tile([16, 128], dt.float32, tag="aw",
                                 name=f"aw{b}_{h}")
                nc.vector.scalar_tensor_tensor(
                    aw32[:], pbuf[:, 0:128], carry32[:], p32[:],
                    op0=Alu.mult, op1=Alu.mult)
                nc.sync.dma_start(
                    aw_ap[b, h * 2048:(h + 1) * 2048, 0].rearrange(
                        "(q f) -> q f", q=16), aw32[:])
                awT_ps = ps_sm.tile([128, 16], dt.float32, tag="sm",
                                    name=f"awT_ps{b}_{h}")
                nc.tensor.transpose(awT_ps[:], aw32[:], ident[0:16, 0:16])
                awT = rowp.tile([128, 16], dt.float32r, tag="awT",
                                name=f"awT{b}_{h}")
                nc.vector.tensor_copy(awT[:], awT_ps[:])
                if h == 0:
                    cvps[b] = ps_cv.tile([1, 512], dt.float32, tag="cv",
                                         name=f"cvps{b}")
                cvp = cvps[b]
                for q2 in range(16):
                    nc.tensor.matmul(cvp[:], awT[:, q2:q2 + 1],
                                     vals[b][h * 16 + q2][:],
                                     start=(h == 0 and q2 == 0),
                                     stop=(h == 1 and q2 == 15))
                if h == 1:
                    cvs = rowp.tile([1, 512], dt.float32, tag="cvs",
                                    name=f"cvs{b}")
                    nc.vector.tensor_copy(cvs[:], cvp[:])
                    nc.sync.dma_start(cv_ap[b:b + 1, 0, :], cvs[:])

            # row 0 projections; start row 1 before row 0's scan so the PE
            # has independent work while the scan chain runs.  DMA issue runs
            # LOOKAHEAD tiles ahead of compute so SDMA never starves the PE.
            di = LOOKAHEAD
            done = 0
            for b, t in tiles:
                proj_tile(b, t)
                if di < len(tiles):
                    issue_key_dmas(*tiles[di])
                    di += 1
                vi = done - VAL_LAG + LOOKAHEAD
                if 0 <= vi < len(tiles):
                    issue_val_dmas(*tiles[vi])
                done += 1
                if done == 2:
                    qb_chain(1)
                if done == 4:
                    flush_mm2()
                    scan_cv_half(0, 0)
                if done == NT + 2:
                    flush_mm2()
                    scan_cv_half(0, 1)
                if done == 15:
                    flush_mm2()
                    scan_cv_half(1, 0)
            for vi in range(len(tiles) - VAL_LAG + LOOKAHEAD, len(tiles)):
                if vi >= 0:
                    issue_val_dmas(*tiles[vi])
            flush_mm2()
            scan_cv_half(1, 1)

    nc.finalize()
    return nc


def _get_nc():
    global _NC
    if _NC is None:
        _NC = _build()
    return _NC


def _shard_inputs(inputs):
    f32 = lambda x: np.ascontiguousarray(np.asarray(x), dtype=np.float32)
    full = {k: f32(v) for k, v in inputs.items()}
    in_maps = []
    for g in range(N_CORES):
        sl = slice(g * BL, (g + 1) * BL)
        in_maps.append({
            "key_enc": full["key_enc"][sl],
            "value": full["value"][sl],
            "query": full["query"][sl],
            "noise": full["noise"][sl],
            "wk_w": full["wk_w"],
            "wk_b": full["wk_b"],
            "wq_w": full["wq_w"],
            "v_v": full["v_v"],
            "v_g": full["v_g"],
            "r": full["r"],
        })
    return in_maps


def run_sharded(inputs, **kw):
    """Run on 8 cores; returns (cv, aw) full outputs + BassKernelResults."""
    nc = _get_nc()
    res = run_bass_kernel_spmd(nc, _shard_inputs(inputs),
                               core_ids=list(range(N_CORES)), **kw)
    cv = np.concatenate([res.results[g]["cv"] for g in range(N_CORES)], axis=0)
    aw = np.concatenate([res.results[g]["aw"] for g in range(N_CORES)], axis=0)
    return cv, aw, res


def kernel(**inputs):
    cv, aw, _ = run_sharded(inputs)
    return cv, aw


# revision 46
# speedup vs baseline: 1.0029x; 1.0003x over previous
"""MoChA (monotonic chunkwise attention, parallel mode) Trainium2 Bass kernel.

Problem: nn_MoChA_88596585382101 (sparse_attention, 8 cores, ridge regime).

Math (per batch row b):
    k    = key_enc @ wk_w.T + wk_b          [K, A]
    q    = query @ wq_w.T                   [A]
    v_w  = (v_g * v_v / ||v_v||)            [A]
    e    = tanh(k + q) . v_w + r            [K]
    p    = sigmoid(e + noise)               [K]
    # reference: cp = exp(cumsum([1, log(clip(1-p,EPS,1))[:-1]]))
    #            aw = p * cp * cumsum(onehot0 / clip(cp,EPS,1))
    # the third factor is identically 1 (cp[0]=e^1 clips to 1.0), so
    #            aw[j] = e^1 * p[j] * prod_{i<j} clip(1-p[i], EPS, 1)
    cv   = aw @ value                       [DV]

Sharding: data-parallel over batch B=16 across 8 cores (2 rows/core),
params replicated; full inputs sharded host-side, outputs concatenated.

Per-core pipeline (per 512-seq tile, fully unrolled under Tile):
  - key_enc tiles PE-transposed ([seq,dk]->[dk,seq], fp32, 2cyc/row) via a
    PSUM stage, DVE-copied to SBUF as float32r,
  - mm1: k^T[a,seq] = wk_w^T-chunks (stationary, f32r) x keyT (moving,
    N=512, f32r streams 1 cyc/row), 4 dk-chunks accumulated in PSUM,
  - tanh(k^T + qb[a]) fused on ACT straight out of PSUM (per-partition
    bias = wq_w @ query + wk_b, computed once on DVE via accum_out),
  - mm2: e[1,seq] = v_w (stationary) x tanh (moving f32r), 4 a-chunks
    accumulated; deferred one a-chunk behind mm1 so the PE never waits,
  - scan, per half-row (2048 seq) so it overlaps later projections:
    e repartitioned [1,2048]->[16,128] with two PE-transpose hops, then
    z = e + r + noise, p = sigmoid(z), q = max(1-p, EPS) on ACT/DVE, and
    the cumulative product of q via DVE tensor_tensor_scan (exclusive via
    a 129-wide buffer with a leading 1); cross-partition carries from a
    [16]->[1,16] PE transpose + a second scan seeded with e^1 (halves
    chained through the carry buffer tail),
  - mm3: cv[1,512] += aw columns (stationary f32r) x value chunks
    (moving f32r, cast on SWDGE load), 32 chunks accumulated in PSUM.

DMA: keys + weights on the two HWDGE queues (sync + scalar), values /
small params on the SWDGE queue, key issue runs 4 tiles ahead and value
issue lags 4 tiles to keep early bandwidth for keys.

float32r = fp32 bits with ~12-bit-mantissa rounding on the PE streaming
path (measured); end-to-end vs the fp32 reference: aw rel-err ~4e-6,
cv rel-err ~1.4e-4 (mm3 operand rounding).

Cost-model timeline: ~134.7 us/core (HBM floor ~100 us, PE busy ~116 us).
"""

import numpy as np
from contextlib import ExitStack

import concourse.bacc as bacc
import concourse.mybir as mybir
import concourse.tile as tile
import concourse.masks as masks
from concourse.bass_utils import run_bass_kernel_spmd

dt = mybir.dt
Act = mybir.ActivationFunctionType
Alu = mybir.AluOpType

N_CORES = 8
B, KMAX, DK, A, DV = 16, 4096, 512, 512, 512
BL = B // N_CORES          # batch rows per core
NT = KMAX // 512           # seq tiles of 512 per row
EPS = 1e-10
E1 = float(np.exp(1.0))

_NC = None


def _build():
    nc = bacc.Bacc("TRN2", target_bir_lowering=False, debug=False,
                   num_devices=N_CORES)
    key_ap = nc.dram_tensor("key_enc", [BL, KMAX, DK], dt.float32,
                            kind="ExternalInput").ap()
    val_ap = nc.dram_tensor("value", [BL, KMAX, DV], dt.float32,
                            kind="ExternalInput").ap()
    qry_ap = nc.dram_tensor("query", [BL, 1, DK], dt.float32,
                            kind="ExternalInput").ap()
    noi_ap = nc.dram_tensor("noise", [BL, KMAX], dt.float32,
                            kind="ExternalInput").ap()
    wkw_ap = nc.dram_tensor("wk_w", [A, DK], dt.float32,
                            kind="ExternalInput").ap()
    wkb_ap = nc.dram_tensor("wk_b", [A], dt.float32,
                            kind="ExternalInput").ap()
    wqw_ap = nc.dram_tensor("wq_w", [A, DK], dt.float32,
                            kind="ExternalInput").ap()
    vv_ap = nc.dram_tensor("v_v", [1, A], dt.float32,
                           kind="ExternalInput").ap()
    vg_ap = nc.dram_tensor("v_g", [1], dt.float32,
                           kind="ExternalInput").ap()
    r_ap = nc.dram_tensor("r", [1], dt.float32, kind="ExternalInput").ap()
    cv_ap = nc.dram_tensor("cv", [BL, 1, DV], dt.float32,
                           kind="ExternalOutput").ap()
    aw_ap = nc.dram_tensor("aw", [BL, KMAX, 1], dt.float32,
                           kind="ExternalOutput").ap()

    with tile.TileContext(nc) as tc:
        with ExitStack() as ctx:
            pre = ctx.enter_context(tc.tile_pool(name="pre", bufs=1))
            keyp = ctx.enter_context(tc.tile_pool(name="keyp", bufs=4))
            ktp = ctx.enter_context(tc.tile_pool(name="ktp", bufs=3))
            thp = ctx.enter_context(tc.tile_pool(name="thp", bufs=2))
            valp = ctx.enter_context(tc.tile_pool(name="valp", bufs=29))
            rowp = ctx.enter_context(tc.tile_pool(name="rowp", bufs=2))
            ps_mm1 = ctx.enter_context(
                tc.tile_pool(name="ps_mm1", bufs=3, space="PSUM"))
            ps_stage = ctx.enter_context(
                tc.tile_pool(name="ps_stage", bufs=3, space="PSUM"))
            ps_cv = ctx.enter_context(
                tc.tile_pool(name="ps_cv", bufs=1, space="PSUM"))
            ps_sm = ctx.enter_context(
                tc.tile_pool(name="ps_sm", bufs=1, space="PSUM"))

            ident = pre.tile([128, 128], dt.float32)
            masks.make_identity(nc, ident[:])
            # PE warm-up: keep the array busy (pstate/HAM ramp) while the
            # first wk/key DMAs are in flight; results are never read.
            warm_ps = ps_sm.tile([128, 128], dt.float32, tag="sm",
                                 name="warm_ps")
            for _ in range(10):
                nc.tensor.transpose(warm_ps[:], ident[:], ident[:])

            # ---- small parameter loads (SWDGE; v_v first for the norm) ----
            vv_sb = pre.tile([1, A], dt.float32)
            nc.gpsimd.dma_start(vv_sb[:], vv_ap)
            vg_sb = pre.tile([1, 1], dt.float32)
            nc.gpsimd.dma_start(vg_sb[:], vg_ap.unsqueeze(0))
            r_sb = pre.tile([1, 1], dt.float32)
            nc.gpsimd.dma_start(r_sb[:], r_ap.unsqueeze(0))
            qry_row = pre.tile([1, BL * 512], dt.float32)
            nc.gpsimd.dma_start(
                qry_row[:], qry_ap.rearrange("b one d -> one (b d)"))
            wkb_sb = pre.tile([128, 4], dt.float32)
            nc.gpsimd.dma_start(wkb_sb[:], wkb_ap.rearrange("(c p) -> p c", p=128))
            # ---- v_w scalar chain (early: ACT sqrt before the tanh set) ----
            sq = pre.tile([1, A], dt.float32)
            nc.vector.tensor_mul(sq[:], vv_sb[:], vv_sb[:])
            ss = pre.tile([1, 1], dt.float32)
            nc.vector.reduce_sum(ss[:], sq[:], axis=mybir.AxisListType.X)
            s0 = pre.tile([1, 1], dt.float32)
            nc.scalar.sqrt(s0[:], ss[:])
            rt0 = pre.tile([1, 1], dt.float32)
            nc.vector.reciprocal(rt0[:], s0[:])
            s1 = pre.tile([1, 1], dt.float32)
            nc.vector.tensor_mul(s1[:], ss[:], rt0[:])
            nc.vector.tensor_add(s1[:], s1[:], s0[:])
            nc.vector.tensor_scalar_mul(s1[:], s1[:], 0.5)
            inv = pre.tile([1, 1], dt.float32)
            nc.vector.reciprocal(inv[:], s1[:])
            scl = pre.tile([1, 1], dt.float32)
            nc.vector.tensor_mul(scl[:], inv[:], vg_sb[:])
            vw_row = pre.tile([1, A], dt.float32)
            nc.vector.tensor_scalar_mul(vw_row[:], vv_sb[:], scl[0:1, 0:1])
            # ---- wk_w^T and wq_w^T (PE transpose of 128x128 blocks) ----
            def load_transposed(w_ap, out_dtype, nm):
                chunks = []
                for c in range(4):
                    t_ = keyp.tile([128, 512], dt.float32, tag=f"key{c}",
                                   name=f"{nm}_ld{c}")
                    eng = nc.sync if c < 2 else nc.scalar
                    eng.dma_start(t_[:], w_ap[c * 128:(c + 1) * 128, :])
                    chunks.append(t_)
                outs = [pre.tile([128, 512], out_dtype, name=f"{nm}T{d}")
                        for d in range(4)]
                for c in (0, 2, 1, 3):  # DMA arrival order (2 queues)
                    stg = ps_stage.tile([128, 512], dt.float32, tag="stg",
                                        name=f"{nm}_cstg{c}")
                    for d in range(4):
                        nc.tensor.transpose(
                            stg[:, d * 128:(d + 1) * 128],
                            chunks[c][:, d * 128:(d + 1) * 128], ident[:])
                    for d in range(4):
                        nc.vector.tensor_copy(
                            outs[d][:, c * 128:(c + 1) * 128],
                            stg[:, d * 128:(d + 1) * 128])
                return outs

            wkT = load_transposed(wkw_ap, dt.float32r, "wk")



            vw_sb = pre.tile([128, 4], dt.float32r)
            for c in range(4):
                tp = ps_sm.tile([128, 32], dt.float32, tag="sm",
                                name=f"vwT{c}")
                nc.tensor.transpose(tp[:, 0:1],
                                    vw_row[0:1, c * 128:(c + 1) * 128],
                                    ident[0:1, 0:1])
                nc.vector.tensor_copy(vw_sb[:, c:c + 1], tp[:, 0:1])

            # ---- qb[b] = wq_w @ query[b] + wk_b ----
            # query broadcast along partitions (ones outer product on PE),
            # then one fused multiply+row-sum (accum_out) per (b, a-chunk).
            ones_col = pre.tile([1, 128], dt.float32)
            nc.vector.memset(ones_col[:], 1.0)
            qb_sb = pre.tile([128, BL * 4], dt.float32)
            qtmp = pre.tile([128, 512], dt.float32)
            qbcast = pre.tile([128, 512], dt.float32)
            def qb_chain(b):
                qbc_ps = ps_stage.tile([128, 512], dt.float32, tag="stg",
                                       name=f"qbc_ps{b}")
                nc.tensor.matmul(qbc_ps[:], ones_col[:],
                                 qry_row[0:1, b * 512:(b + 1) * 512],
                                 start=True, stop=True)
                nc.vector.tensor_copy(qbcast[:], qbc_ps[:])
                for c in range(4):
                    nc.vector.scalar_tensor_tensor(
                        qtmp[:], wq_sb[c][:], 0.0, qbcast[:],
                        op0=Alu.add, op1=Alu.mult,
                        accum_out=qb_sb[:, 4 * b + c:4 * b + c + 1])
                for c in range(4):
                    nc.vector.tensor_add(qb_sb[:, 4 * b + c:4 * b + c + 1],
                                         qb_sb[:, 4 * b + c:4 * b + c + 1],
                                         wkb_sb[:, c:c + 1])



            erow = {}
            noise32s = {}
            pbufs = {}
            cbufs = {}
            for b in range(BL):
                for h in range(2):
                    pbufs[(b, h)] = rowp.tile([16, 129], dt.float32,
                                              tag="pbuf", bufs=4,
                                              name=f"pbuf{b}_{h}")
                    nc.vector.memset(pbufs[(b, h)][:, 0:1], 1.0)
                    cbufs[(b, h)] = rowp.tile([1, 17], dt.float32,
                                              tag="cbuf", bufs=4,
                                              name=f"cbuf{b}_{h}")
                    if h == 0:
                        nc.vector.memset(cbufs[(b, h)][0:1, 0:1], E1)
            vals = {b: [] for b in range(BL)}
            for b in range(BL):
                erow[b] = rowp.tile([1, KMAX], dt.float32, tag="erow",
                                    name=f"erow{b}")
                for h in range(2):
                    noise32s[(b, h)] = rowp.tile(
                        [16, 128], dt.float32, tag="noi", bufs=4,
                        name=f"noi{b}_{h}")
                    nc.gpsimd.dma_start(
                        noise32s[(b, h)][:],
                        noi_ap[b:b + 1, h * 2048:(h + 1) * 2048].rearrange(
                            "one (q f) -> q f", q=16))
            pending = {}
            pend_mm2 = [None]

            def flush_mm2():
                if pend_mm2[0] is not None:
                    fn = pend_mm2[0]
                    pend_mm2[0] = None
                    fn()

            def issue_key_dmas(b, t):
                base = t * 512
                ksb = []
                for j in range(4):
                    kt = keyp.tile([128, 512], dt.float32, tag=f"key{j}",
                                   name=f"key{b}_{t}_{j}")
                    nc.sync.dma_start(
                        kt[:],
                        key_ap[b, base + j * 128:base + (j + 1) * 128, :])
                    ksb.append(kt)
                pending[(b, t)] = ksb

            def issue_val_dmas(b, t):
                for cc in range(4):
                    idx = t * 4 + cc
                    vt = valp.tile([128, 512], dt.float32r, tag="val",
                                   name=f"val{b}_{idx}")
                    nc.gpsimd.dma_start(
                        vt[:], val_ap[b, idx * 128:(idx + 1) * 128, :])
                    vals[b].append(vt)

            tiles = [(0, t) for t in range(NT)] + [(1, 0), (1, 1)] \
                + [(1, t) for t in range(2, NT)]
            LOOKAHEAD = 4
            VAL_LAG = 4
            issue_key_dmas(*tiles[0])
            # wq_w loads sit behind tile-0's keys on the sync queue: they
            # only feed the qb chain, which isn't needed until tanh(t0)
            wq_sb = []
            for c in range(4):
                t_ = pre.tile([128, 512], dt.float32, name=f"wq_ld{c}")
                nc.sync.dma_start(t_[:], wqw_ap[c * 128:(c + 1) * 128, :])
                wq_sb.append(t_)
            for i in range(1, LOOKAHEAD):
                issue_key_dmas(*tiles[i])
            qb_chain(0)

            # ---- r broadcast to [32,1]; ones tiles ----
            ones_r = pre.tile([1, 32], dt.float32)
            nc.vector.memset(ones_r[:], 1.0)
            r32_ps = ps_sm.tile([128, 32], dt.float32, tag="sm", name="r32ps")
            nc.tensor.matmul(r32_ps[0:32, 0:1], ones_r[:], r_sb[:],
                             start=True, stop=True)
            r32_sb = pre.tile([32, 1], dt.float32)
            nc.vector.tensor_copy(r32_sb[:], r32_ps[0:32, 0:1])
            ones32 = pre.tile([32, 128], dt.float32)
            nc.vector.memset(ones32[:], 1.0)



            def proj_tile(b, t):
                """One 512-seq tile of the projection phase for row b."""
                base = t * 512
                ksb = pending.pop((b, t))
                kT = []
                for d in range(4):
                    stg = ps_stage.tile([128, 512], dt.float32, tag="stg",
                                        name=f"stg{b}_{t}_{d}")
                    for j in range(4):
                        nc.tensor.transpose(
                            stg[:, j * 128:(j + 1) * 128],
                            ksb[j][:, d * 128:(d + 1) * 128], ident[:])
                    kt_ = ktp.tile([128, 512], dt.float32r, tag=f"kT{d}",
                                   name=f"kT{b}_{t}_{d}")
                    nc.vector.tensor_copy(kt_[:], stg[:])
                    kT.append(kt_)
                if len(kT) == 4:
                    flush_mm2()
                e_ps = ps_sm.tile([1, 512], dt.float32, tag="sm",
                                 name=f"e_ps{b}_{t}")
                ths = []
                for c in range(4):
                    kp = ps_mm1.tile([128, 512], dt.float32, tag="mm1",
                                     name=f"k_ps{b}_{t}_{c}")
                    for d in range(4):
                        nc.tensor.matmul(kp[:],
                                         wkT[d][:, c * 128:(c + 1) * 128],
                                         kT[d][:],
                                         start=(d == 0), stop=(d == 3))
                    th = thp.tile([128, 512], dt.float32r, tag=f"th{c}",
                                  name=f"th{b}_{t}_{c}")
                    nc.scalar.activation(th[:], kp[:], Act.Tanh,
                                         bias=qb_sb[:, 4 * b + c:4 * b + c + 1])
                    ths.append(th)
                    if c >= 1:  # defer mm2 one a-chunk so PE never waits ACT
                        cm = c - 1
                        nc.tensor.matmul(e_ps[:], vw_sb[:, cm:cm + 1],
                                         ths[cm][:],
                                         start=(cm == 0), stop=False)

                def _mm2_tail(e_ps=e_ps, th3=ths[3], b=b, base=base):
                    # last a-chunk + e copy, emitted after the NEXT tile's
                    # transposes so the PE never waits on tanh(c3)
                    nc.tensor.matmul(e_ps[:], vw_sb[:, 3:4], th3[:],
                                     start=False, stop=True)
                    nc.scalar.copy(erow[b][0:1, base:base + 512], e_ps[:])
                pend_mm2[0] = _mm2_tail

            cvps = {}

            def scan_cv_half(b, h):
                """Scan chain + cv contraction for row b, half h (2048 seq)."""
                ecols_ps = ps_sm.tile([128, 16], dt.float32, tag="sm",
                                      name=f"ecols_ps{b}_{h}")
                for q in range(16):
                    qq = h * 16 + q
                    nc.tensor.transpose(ecols_ps[:, q:q + 1],
                                        erow[b][0:1, qq * 128:(qq + 1) * 128],
                                        ident[0:1, 0:1])
                ecols = rowp.tile([128, 16], dt.float32, tag="ecols",
                                  name=f"ecols{b}_{h}")
                nc.vector.tensor_copy(ecols[:], ecols_ps[:])
                e32_ps = ps_sm.tile([16, 128], dt.float32, tag="sm",
                                    name=f"e32_ps{b}_{h}")
                nc.tensor.transpose(e32_ps[:], ecols[:], ident[:])
                noise32 = noise32s[(b, h)]
                z32 = rowp.tile([16, 128], dt.float32, tag="z32",
                                name=f"z32_{b}_{h}")
                nc.vector.scalar_tensor_tensor(
                    z32[:], e32_ps[:], r32_sb[0:16, :], noise32[:],
                    op0=Alu.add, op1=Alu.add)
                p32 = rowp.tile([16, 128], dt.float32, tag="p32",
                                name=f"p32_{b}_{h}")
                nc.scalar.activation(p32[:], z32[:], Act.Sigmoid)
                qc = rowp.tile([16, 128], dt.float32, tag="qc",
                               name=f"qc{b}_{h}")
                nc.vector.scalar_tensor_tensor(
                    qc[:], p32[:], -1.0, ones32[0:16, :],
                    op0=Alu.mult, op1=Alu.add)
                nc.vector.tensor_scalar_max(qc[:], qc[:], EPS)
                pbuf = pbufs[(b, h)]
                nc.vector.tensor_tensor_scan(
                    pbuf[:, 1:129], qc[:], ones32[0:16, :], 1.0,
                    op0=Alu.mult, op1=Alu.mult)
                # cross-partition carries; chain across halves via cbuf[16]
                totT_ps = ps_sm.tile([1, 16], dt.float32, tag="sm",
                                     name=f"totT{b}_{h}")
                nc.tensor.transpose(totT_ps[:], pbuf[:, 128:129],
                                    ident[0:16, 0:16])
                cbuf = cbufs[(b, h)]
                init = E1 if h == 0 else cbufs[(b, 0)][0:1, 16:17]
                if h == 1:
                    nc.vector.tensor_copy(cbuf[0:1, 0:1],
                                          cbufs[(b, 0)][0:1, 16:17])
                nc.vector.tensor_tensor_scan(
                    cbuf[0:1, 1:17], totT_ps[:], ones_r[0:1, 0:16], init,
                    op0=Alu.mult, op1=Alu.mult)
                carryT_ps = ps_sm.tile([16, 1], dt.float32, tag="sm",
                                       name=f"carryT{b}_{h}")
                nc.tensor.transpose(carryT_ps[:], cbuf[0:1, 0:16],
                                    ident[0:1, 0:1])
                carry32 = rowp.tile([16, 1], dt.float32, tag="carry",
                                    name=f"carry{b}_{h}")
                nc.vector.tensor_copy(carry32[:], carryT_ps[:])
                aw32 = rowp.tile([16, 128], dt.float32, tag="aw",
                                 name=f"aw{b}_{h}")
                nc.vector.scalar_tensor_tensor(
                    aw32[:], pbuf[:, 0:128], carry32[:], p32[:],
                    op0=Alu.mult, op1=Alu.mult)
                nc.sync.dma_start(
                    aw_ap[b, h * 2048:(h + 1) * 2048, 0].rearrange(
                        "(q f) -> q f", q=16), aw32[:])
                awT_ps = ps_sm.tile([128, 16], dt.float32, tag="sm",
                                    name=f"awT_ps{b}_{h}")
                nc.tensor.transpose(awT_ps[:], aw32[:], ident[0:16, 0:16])
                awT = rowp.tile([128, 16], dt.float32r, tag="awT",
                                name=f"awT{b}_{h}")
                if h == 0:
                    cvps[b] = ps_cv.tile([1, 512], dt.float32, tag="cv",
                                         name=f"cvps{b}")
                cvp = cvps[b]
                # copy awT in two halves so the cv matmuls start as soon as
                # the first 8 columns land
                for half in range(2):
                    nc.vector.tensor_copy(awT[:, half * 8:(half + 1) * 8],
                                          awT_ps[:, half * 8:(half + 1) * 8])
                    for q2 in range(half * 8, (half + 1) * 8):
                        nc.tensor.matmul(cvp[:], awT[:, q2:q2 + 1],
                                         vals[b][h * 16 + q2][:],
                                         start=(h == 0 and q2 == 0),
                                         stop=(h == 1 and q2 == 15))
                if h == 1:
                    cvs = rowp.tile([1, 512], dt.float32, tag="cvs",
                                    name=f"cvs{b}")
                    nc.vector.tensor_copy(cvs[:], cvp[:])
                    nc.sync.dma_start(cv_ap[b:b + 1, 0, :], cvs[:])

            # row 0 projections; start row 1 before row 0's scan so the PE
            # has independent work while the scan chain runs.  DMA issue runs
            # LOOKAHEAD tiles ahead of compute so SDMA never starves the PE.
            di = LOOKAHEAD
            done = 0
            for b, t in tiles:
                proj_tile(b, t)
                if di < len(tiles):
                    issue_key_dmas(*tiles[di])
                    di += 1
                vi = done - VAL_LAG + LOOKAHEAD
                if 0 <= vi < len(tiles):
                    issue_val_dmas(*tiles[vi])
                done += 1
                if done == 2:
                    qb_chain(1)
                if done == 4:
                    flush_mm2()
                    scan_cv_half(0, 0)
                if done == NT + 2:
                    flush_mm2()
                    scan_cv_half(0, 1)
                if done == 15:
                    flush_mm2()
                    scan_cv_half(1, 0)
            for vi in range(len(tiles) - VAL_LAG + LOOKAHEAD, len(tiles)):
                if vi >= 0:
                    issue_val_dmas(*tiles[vi])
            flush_mm2()
            scan_cv_half(1, 1)

    nc.finalize()
    return nc


def _get_nc():
    global _NC
    if _NC is None:
        _NC = _build()
    return _NC


def _shard_inputs(inputs):
    f32 = lambda x: np.ascontiguousarray(np.asarray(x), dtype=np.float32)
    full = {k: f32(v) for k, v in inputs.items()}
    in_maps = []
    for g in range(N_CORES):
        sl = slice(g * BL, (g + 1) * BL)
        in_maps.append({
            "key_enc": full["key_enc"][sl],
            "value": full["value"][sl],
            "query": full["query"][sl],
            "noise": full["noise"][sl],
            "wk_w": full["wk_w"],
            "wk_b": full["wk_b"],
            "wq_w": full["wq_w"],
            "v_v": full["v_v"],
            "v_g": full["v_g"],
            "r": full["r"],
        })
    return in_maps


def run_sharded(inputs, **kw):
    """Run on 8 cores; returns (cv, aw) full outputs + BassKernelResults."""
    nc = _get_nc()
    res = run_bass_kernel_spmd(nc, _shard_inputs(inputs),
                               core_ids=list(range(N_CORES)), **kw)
    cv = np.concatenate([res.results[g]["cv"] for g in range(N_CORES)], axis=0)
    aw = np.concatenate([res.results[g]["aw"] for g in range(N_CORES)], axis=0)
    return cv, aw, res


def kernel(**inputs):
    cv, aw, _ = run_sharded(inputs)
    return cv, aw
